# revision 1
# baseline (speedup 1.0000x reference)
"""Self-contained Trainium2 Bass kernel for nn_DualGATv2 (3-layer GATv2 + MLP).

Sharding: nodes are degree-sorted and snake-dealt across 8 NeuronCores
(graph parallel). Each core owns 6272 table rows (6250 real nodes + 22 pads)
and processes the edges whose *destination* lies in its shard; small weights
are replicated. Projected source features live in a bf16 table per layer
(layer 0 computed fully on every core since x is replicated; layers 1-2 via
AllGather). Per-edge features are fetched with gpsimd dma_gather (int16
indices; two gathers per destination block from overlapping lo/hi windows of
the table to cover >32k rows). Scatter-softmax/scatter-add become dense
per-partition ops: each destination node owns one SBUF partition of its
block, its (band-padded) incoming edges occupy free-dim slots, and an
additive -1e30 mask neutralizes pad slots.
"""
import sys
import numpy as np

sys.path.insert(0, '/opt/trn_rl_repo')

import concourse.bass as bass
import concourse.bacc as bacc
import concourse.tile as tile
from concourse import mybir, library_config
from concourse import bass_utils
from concourse._compat import cdiv

F32 = mybir.dt.float32
BF16 = mybir.dt.bfloat16
I16 = mybir.dt.int16
AL = mybir.AluOpType
ACTF = mybir.ActivationFunctionType
AX = mybir.AxisListType

NC = 8
P = 128
HID = 32
HEADS = 4
NEG_SLOPE = 0.2
LN_EPS = 1e-5
NEG_BIG = -1.0e30
IDX_WIN = 32768


# ----------------------------------------------------------------------------
# host-side preprocessing
# ----------------------------------------------------------------------------

def _prep(x, edge_index):
    x = np.asarray(x, dtype=np.float32)
    N = x.shape[0]
    src = np.asarray(edge_index[0], dtype=np.int64)
    dst = np.asarray(edge_index[1], dtype=np.int64)
    loop = np.arange(N, dtype=np.int64)
    src = np.concatenate([src, loop])
    dst = np.concatenate([dst, loop])

    deg = np.bincount(dst, minlength=N)

    order = np.argsort(-deg, kind='stable')
    ranks = np.arange(N)
    g, j = ranks // NC, ranks % NC
    core_of_rank = np.where(g % 2 == 0, j, NC - 1 - j)
    core = np.zeros(N, dtype=np.int64)
    core[order] = core_of_rank
    # position within core, in rank order
    pos = np.zeros(N, dtype=np.int64)
    cnt = np.zeros(NC, dtype=np.int64)
    for r in range(N):
        n = order[r]
        c = core[n]
        pos[n] = cnt[c]
        cnt[c] += 1

    NSH_REAL = cdiv(N, NC)
    NSH = cdiv(NSH_REAL + 1, P) * P       # ensure >= 1 pad row per core
    NBLK = NSH // P
    TAB = NC * NSH
    row = core * NSH + pos
    W_LO = min(IDX_WIN, TAB)
    HI_BASE = max(0, TAB - IDX_WIN)
    ZR_LO = NSH - 1                        # core 0's last pad row (< W_LO)
    ZR_HI = TAB - 1 - HI_BASE              # last core's last pad, hi-local

    e_order = np.argsort(dst, kind='stable')
    src_s = src[e_order]
    dst_s = dst[e_order]
    starts = np.searchsorted(dst_s, np.arange(N))
    ends = np.searchsorted(dst_s, np.arange(N) + 1)
    rs_all = row[src_s]

    cA = np.zeros(N, dtype=np.int64)
    cB = np.zeros(N, dtype=np.int64)
    edgeA = [None] * N
    edgeB = [None] * N
    for n in range(N):
        s, e = starts[n], ends[n]
        rs = rs_all[s:e]
        d = e - s
        forcedA = rs < HI_BASE
        forcedB = rs >= W_LO
        nAf = int(forcedA.sum())
        nBf = int(forcedB.sum())
        ca = min(max((d + 1) // 2, nAf), d - nBf)
        selA = forcedA.copy()
        nflexA = ca - nAf
        if nflexA > 0:
            fidx = np.nonzero(~forcedA & ~forcedB)[0]
            selA[fidx[:nflexA]] = True
        edgeA[n] = rs[selA]
        edgeB[n] = rs[~selA] - HI_BASE
        cA[n] = ca
        cB[n] = d - ca

    node_at = np.full((NC, NSH), -1, dtype=np.int64)
    node_at[core, pos] = np.arange(N)

    K_A = np.zeros(NBLK, dtype=np.int64)
    K_B = np.zeros(NBLK, dtype=np.int64)
    for b in range(NBLK):
        sl = node_at[:, b * P:(b + 1) * P].reshape(-1)
        sl = sl[sl >= 0]
        if len(sl):
            K_A[b] = cA[sl].max()
            K_B[b] = cB[sl].max()
    K_A = np.maximum(K_A, 1)
    K_B = np.maximum(K_B, 1)

    SUMKT = int((K_A + K_B).sum())
    IDXW = int(8 * SUMKT)
    idx_all = np.zeros((NC, P, IDXW), dtype=np.int16)
    mask_all = np.full((NC, P, SUMKT), NEG_BIG, dtype=np.float32)

    def wrap(flat):
        n = len(flat)
        S = cdiv(n, 16)
        a = np.zeros(16 * S, np.int16)
        a[:n] = flat
        return np.tile(a.reshape(S, 16).T, (8, 1))

    icol = 0
    mcol = 0
    for b in range(NBLK):
        ka, kb = int(K_A[b]), int(K_B[b])
        for c in range(NC):
            flatA = np.full(ka * P, ZR_LO, np.int64)
            flatB = np.full(kb * P, ZR_HI, np.int64)
            for p in range(P):
                n = node_at[c, b * P + p]
                if n < 0:
                    continue
                ea, eb = edgeA[n], edgeB[n]
                la, lb = len(ea), len(eb)
                if la:
                    flatA[np.arange(la) * P + p] = ea
                    mask_all[c, p, mcol:mcol + la] = 0.0
                if lb:
                    flatB[np.arange(lb) * P + p] = eb
                    mask_all[c, p, mcol + ka:mcol + ka + lb] = 0.0
            idx_all[c, :, icol:icol + 8 * ka] = wrap(flatA.astype(np.int16))
            idx_all[c, :, icol + 8 * ka:icol + 8 * (ka + kb)] = \
                wrap(flatB.astype(np.int16))
        icol += 8 * (ka + kb)
        mcol += ka + kb

    IND = x.shape[1]
    xT = np.zeros((IND, TAB), dtype=np.float32)
    xT[:, row] = x.T
    xT_own = np.ascontiguousarray(
        xT.reshape(IND, NC, NSH).transpose(1, 0, 2))   # [NC, IND, NSH]

    padmask = (np.arange(P) < (NSH_REAL - (NBLK - 1) * P)) \
        .astype(np.float32).reshape(P, 1)
    st = dict(N=N, NSH=NSH, NSH_REAL=NSH_REAL, NBLK=NBLK, TAB=TAB,
              W_LO=W_LO, HI_BASE=HI_BASE, K_A=K_A.tolist(),
              K_B=K_B.tolist(), SUMKT=SUMKT, IDXW=IDXW, IN_DIM=IND)
    return st, xT, xT_own, idx_all, mask_all, row, padmask


def _rep(v):
    v = np.asarray(v, dtype=np.float32).reshape(1, -1)
    return np.ascontiguousarray(np.tile(v, (P, 1)))


# ----------------------------------------------------------------------------
# kernel builder
# ----------------------------------------------------------------------------

def _build(st):
    import os
    STAGE = os.environ.get('STAGE', 'FULL')
    NSH, NBLK, TAB = st['NSH'], st['NBLK'], st['TAB']
    NSH_REAL = st['NSH_REAL']
    W_LO, HI_BASE = st['W_LO'], st['HI_BASE']
    K_A, K_B = st['K_A'], st['K_B']
    IDXW, SUMKT = st['IDXW'], st['SUMKT']
    IND = st['IN_DIM']
    NT = TAB // P
    PAD_P0 = NSH_REAL - (NBLK - 1) * P     # first pad partition in last block

    LCFG = [(HEADS, HID, HEADS * HID, IND),
            (HEADS, HID, HEADS * HID, HEADS * HID),
            (1, HID, HID, HEADS * HID)]

    nc = bacc.Bacc('TRN2', target_bir_lowering=False, debug=False,
                   enable_asserts=True, num_devices=NC,
                   num_swdge_queues=4)

    def ein(name, shape, dt=F32):
        return nc.dram_tensor(name, shape, dt, kind='ExternalInput')

    xT_d = ein('xT', [IND, TAB])
    xTo_d = ein('xT_own', [IND, NSH])
    idx_d = ein('idx_all', [P, IDXW], I16)
    pmask_d = ein('padmask', [P, 1])
    mask_d = ein('mask_all', [P, SUMKT])
    W01_d = [ein('W01_0', [IND, 256]), ein('W01_1', [128, 256]),
             ein('W01_2', [128, 64])]
    BL01_d = [ein('bl01_0', [P, 256]), ein('bl01_1', [P, 256]),
              ein('bl01_2', [P, 64])]
    ATT_d = [ein('att_0', [P, 128]), ein('att_1', [P, 128]),
             ein('att_2', [P, 32])]
    GG_d = [ein('g_0', [P, 128]), ein('g_1', [P, 128]), ein('g_2', [P, 32])]
    BE_d = [ein('be_0', [P, 128]), ein('be_1', [P, 128]), ein('be_2', [P, 32])]
    BO_d = [ein('bo_0', [P, 128]), ein('bo_1', [P, 128]), ein('bo_2', [P, 32])]
    cW1_d = ein('cW1', [32, 16])
    cb1_d = ein('cb1', [P, 16])
    cW2_d = ein('cW2', [16, 1])
    ident_d = ein('ident', [P, P])
    cb2_d = ein('cb2', [P, 1])
    out_d = nc.dram_tensor('out', [NSH], F32, kind='ExternalOutput')

    tabs = [nc.dram_tensor('table0', [TAB, 128], BF16, kind='Internal'),
            nc.dram_tensor('table1', [TAB, 128], BF16, kind='Internal',
                           addr_space='Shared'),
            nc.dram_tensor('table2', [TAB, 128], BF16, kind='Internal',
                           addr_space='Shared')]
    ag_in = [None,
             nc.dram_tensor('ag_in1', [NSH, 128], BF16, kind='Internal'),
             nc.dram_tensor('ag_in2', [NSH, 128], BF16, kind='Internal')]

    import contextlib
    with tile.TileContext(nc) as tc, contextlib.ExitStack() as ctx:
        cpool = ctx.enter_context(tc.tile_pool(name='consts', bufs=1))
        gpool = ctx.enter_context(tc.tile_pool(name='g', bufs=3))
        tpool = ctx.enter_context(tc.tile_pool(name='t', bufs=2))
        spool = ctx.enter_context(tc.tile_pool(name='small', bufs=3))
        npool = ctx.enter_context(tc.tile_pool(name='node', bufs=2))
        hpool = ctx.enter_context(tc.tile_pool(name='h', bufs=1))
        xpool = ctx.enter_context(tc.tile_pool(name='xt', bufs=6))
        stpool = ctx.enter_context(tc.tile_pool(name='stage', bufs=4))
        pspool = ctx.enter_context(tc.tile_pool(name='ps', bufs=3,
                                                space='PSUM'))
        ps2pool = ctx.enter_context(tc.tile_pool(name='ps2', bufs=2,
                                                 space='PSUM'))

        def load_const(dram, shape, dt=F32):
            t = cpool.tile(shape, dt, tag='c_' + dram.name,
                           name='c_' + dram.name)
            nc.sync.dma_start(out=t[:], in_=dram[:])
            return t

        ident = load_const(ident_d, [P, P])
        W01_s = [load_const(W01_d[l], list(W01_d[l].shape)) for l in range(3)]
        BL01_s = [load_const(BL01_d[l], list(BL01_d[l].shape)) for l in range(3)]
        GG_s = [load_const(GG_d[l], list(GG_d[l].shape)) for l in range(3)]
        BE_s = [load_const(BE_d[l], list(BE_d[l].shape)) for l in range(3)]
        BO_s = [load_const(BO_d[l], list(BO_d[l].shape)) for l in range(3)]
        cW1_s = load_const(cW1_d, [32, 16])
        cb1_s = load_const(cb1_d, [P, 16])
        cW2_s = load_const(cW2_d, [16, 1])
        cb2_s = load_const(cb2_d, [P, 1])
        attb = []
        for l in range(3):
            f = load_const(ATT_d[l], ATT_d[l].shape)
            t = cpool.tile(ATT_d[l].shape, BF16, tag=f'attb{l}',
                           name=f'attb{l}')
            nc.vector.tensor_copy(out=t[:], in_=f[:])
            attb.append(t)

        pmask_s = load_const(pmask_d, [P, 1])
        eps_t = cpool.tile([P, 1], F32, tag='eps', name='eps')
        nc.vector.memset(eps_t[:], float(LN_EPS))
        mask_s = cpool.tile([P, SUMKT], F32, tag='mask')
        nc.sync.dma_start(out=mask_s[:], in_=mask_d[:])

        h_res = [hpool.tile([P, NBLK * 128], F32, tag='h0', name='h0'),
                 hpool.tile([P, NBLK * 128], F32, tag='h1', name='h1'),
                 hpool.tile([P, NBLK * HID], F32, tag='h2', name='h2')]
        xr_res = [hpool.tile([P, NBLK * 128], BF16, tag='xr0', name='xr0'),
                  hpool.tile([P, NBLK * 128], BF16, tag='xr1', name='xr1'),
                  hpool.tile([P, NBLK * HID], BF16, tag='xr2', name='xr2')]
        out_sb = hpool.tile([P, NBLK], F32, tag='outsb')

        # ------------- layer-0 projections (x replicated => local) -------
        for t in range(NT):
            xt = xpool.tile([IND, P], F32, tag='xt')
            nc.scalar.dma_start(out=xt[:], in_=xT_d[:, t * P:(t + 1) * P])
            ps = pspool.tile([P, 256], F32, tag='psA')
            nc.tensor.matmul(out=ps[:, 0:128], lhsT=xt[:],
                             rhs=W01_s[0][:, 0:128], start=True, stop=True)
            stg = stpool.tile([P, 128], BF16, tag='stgA')
            nc.vector.tensor_tensor(out=stg[:], in0=ps[:, 0:128],
                                    in1=BL01_s[0][:, 0:128], op=AL.add)
            if t % NBLK == NBLK - 1:
                nc.vector.tensor_scalar_mul(stg[:], stg[:], pmask_s[:])
            nc.sync.dma_start(out=tabs[0][t * P:(t + 1) * P, :], in_=stg[:])
        for b in range(NBLK):
            xo = xpool.tile([IND, P], F32, tag='xo')
            nc.scalar.dma_start(out=xo[:], in_=xTo_d[:, b * P:(b + 1) * P])
            ps = pspool.tile([P, 256], F32, tag='psA')
            nc.tensor.matmul(out=ps[:, 0:128], lhsT=xo[:],
                             rhs=W01_s[0][:, 128:256], start=True, stop=True)
            nc.vector.tensor_tensor(out=xr_res[0][:, b * 128:(b + 1) * 128],
                                    in0=ps[:, 0:128],
                                    in1=BL01_s[0][:, 128:256], op=AL.add)

        # cumulative idx/mask offsets per block
        ic_of = []
        mc_of = []
        ic = mc = 0
        for b in range(NBLK):
            ic_of.append(ic)
            mc_of.append(mc)
            ic += 8 * (K_A[b] + K_B[b])
            mc += K_A[b] + K_B[b]

        qc = [0]

        def edge_block(l, b):
            H, DO, FE, _ = LCFG[l]
            ka, kb = K_A[b], K_B[b]
            kt = ka + kb
            ic, mc = ic_of[b], mc_of[b]
            GA = gpool.tile([P, ka, 128], BF16, tag='GA')
            GB = gpool.tile([P, kb, 128], BF16, tag='GB')
            idx_s = xpool.tile([P, 8 * kt], I16, tag='idxblk')
            nc.scalar.dma_start(out=idx_s[:], in_=idx_d[:, ic:ic + 8 * kt])
            ic = 0
            GMAX = 7  # 7*128 idxs -> 56+sem descriptors per engine packet
            for off in range(0, ka, GMAX):
                kk = min(GMAX, ka - off)
                nc.gpsimd.dma_gather(
                    GA[:, off:off + kk, :], tabs[l][0:W_LO, :],
                    idx_s[:, ic + 8 * off:ic + 8 * (off + kk)],
                    kk * P, kk * P, 128, queue_num=qc[0] % 4)
                qc[0] += 1
            for off in range(0, kb, GMAX):
                kk = min(GMAX, kb - off)
                nc.gpsimd.dma_gather(
                    GB[:, off:off + kk, :], tabs[l][HI_BASE:TAB, :],
                    idx_s[:, ic + 8 * (ka + off):ic + 8 * (ka + off + kk)],
                    kk * P, kk * P, 128, queue_num=qc[0] % 4)
                qc[0] += 1
            AGG = npool.tile([P, FE], F32, tag='AGG')
            T = tpool.tile([P, kt * FE], BF16, tag='T')
            T3 = T[:].rearrange('p (k f) -> p k f', k=kt)
            xr_col = xr_res[l][:, b * FE:(b + 1) * FE]
            nc.vector.tensor_tensor(
                out=T3[:, 0:ka, :], in0=GA[:, :, 0:FE],
                in1=xr_col.unsqueeze(1).to_broadcast([P, ka, FE]), op=AL.add)
            nc.vector.tensor_tensor(
                out=T3[:, ka:kt, :], in0=GB[:, :, 0:FE],
                in1=xr_col.unsqueeze(1).to_broadcast([P, kb, FE]), op=AL.add)
            nc.vector.scalar_tensor_tensor(
                out=T[:], in0=T[:], scalar=float(NEG_SLOPE), in1=T[:],
                op0=AL.mult, op1=AL.max)
            att_b = attb[l][:].unsqueeze(1).to_broadcast([P, kt, FE])
            nc.vector.tensor_tensor(out=T3, in0=T3, in1=att_b, op=AL.mult)
            # k-major logits: LG[p, k*H + h]
            LG = spool.tile([P, kt * H], F32, tag='LG')
            LG3 = LG[:].rearrange('p (k h) -> p k h', h=H)
            LG_hk = LG[:].rearrange('p (k h) -> p h k', h=H)
            T_khd = T[:].rearrange('p (k h d) -> p k h d', h=H, d=DO)
            nc.vector.tensor_reduce(out=LG[:], in_=T_khd, axis=AX.X,
                                    op=AL.add)
            mask_b = mask_s[:, mc:mc + kt].unsqueeze(2) \
                .to_broadcast([P, kt, H])
            nc.vector.tensor_tensor(out=LG3, in0=LG3, in1=mask_b, op=AL.add)
            M = spool.tile([P, H], F32, tag='M')
            nc.vector.tensor_reduce(out=M[:], in_=LG_hk, axis=AX.X,
                                    op=AL.max)
            nc.vector.tensor_tensor(
                out=LG3, in0=LG3,
                in1=M[:].unsqueeze(1).to_broadcast([P, kt, H]),
                op=AL.subtract)
            nc.scalar.activation(out=LG[:], in_=LG[:], func=ACTF.Exp)
            DN = spool.tile([P, H], F32, tag='DN')
            nc.vector.tensor_reduce(out=DN[:], in_=LG_hk, axis=AX.X,
                                    op=AL.add)
            R = spool.tile([P, H], F32, tag='R')
            nc.vector.reciprocal(R[:], DN[:])
            Abf = spool.tile([P, kt * H], BF16, tag='Abf')
            nc.scalar.copy(out=Abf[:], in_=LG[:])
            # weighted messages into T (reused), layout (h, d, k)
            W_out = T[:].rearrange('p (h d k) -> p h d k', h=H, d=DO)
            A3 = Abf[:].rearrange('p (k h) -> p h k', h=H)
            nc.vector.tensor_tensor(
                out=W_out[:, :, :, 0:ka],
                in0=GA[:, :, 0:FE].rearrange('p k (h d) -> p h d k', h=H),
                in1=A3[:, :, 0:ka].unsqueeze(2).to_broadcast([P, H, DO, ka]),
                op=AL.mult)
            nc.vector.tensor_tensor(
                out=W_out[:, :, :, ka:kt],
                in0=GB[:, :, 0:FE].rearrange('p k (h d) -> p h d k', h=H),
                in1=A3[:, :, ka:kt].unsqueeze(2).to_broadcast([P, H, DO, kb]),
                op=AL.mult)
            nc.vector.tensor_reduce(out=AGG[:], in_=W_out, axis=AX.X,
                                    op=AL.add)
            AGG3 = AGG[:].rearrange('p (h d) -> p h d', h=H)
            nc.vector.tensor_tensor(
                out=AGG3, in0=AGG3,
                in1=R[:].unsqueeze(2).to_broadcast([P, H, DO]), op=AL.mult)
            nc.vector.tensor_tensor(out=AGG[:], in0=AGG[:], in1=BO_s[l][:],
                                    op=AL.add)
            return AGG

        def node_tail(l, b, AGG):
            H, DO, FE, _ = LCFG[l]
            if NT == 'off':
                nc.vector.tensor_copy(out=h_res[l][:, b * FE:(b + 1) * FE],
                                      in_=AGG[:])
                return
            SM = npool.tile([P, 1], F32, tag='SM')
            nc.vector.tensor_reduce(out=SM[:], in_=AGG[:], axis=AX.X,
                                    op=AL.add)
            MU = npool.tile([P, 1], F32, tag='MU')
            nc.vector.tensor_scalar_mul(MU[:], SM[:], 1.0 / FE)
            SQJ = npool.tile([P, FE], F32, tag='SQJ')
            SSQ = npool.tile([P, 1], F32, tag='SSQ')
            nc.scalar.activation(out=SQJ[:], in_=AGG[:], func=ACTF.Square,
                                 accum_out=SSQ[:])
            MU2 = npool.tile([P, 1], F32, tag='MU2')
            nc.scalar.activation(out=MU2[:], in_=MU[:], func=ACTF.Square)
            VAR = npool.tile([P, 1], F32, tag='VAR')
            nc.vector.scalar_tensor_tensor(
                out=VAR[:], in0=SSQ[:], scalar=1.0 / FE, in1=MU2[:],
                op0=AL.mult, op1=AL.subtract)
            SD = npool.tile([P, 1], F32, tag='SD')
            nc.scalar.activation(out=SD[:], in_=VAR[:], func=ACTF.Sqrt,
                                 bias=eps_t[:])
            IV = npool.tile([P, 1], F32, tag='IV')
            nc.vector.reciprocal(IV[:], SD[:])
            XH = npool.tile([P, FE], F32, tag='XH')
            nc.vector.scalar_tensor_tensor(
                out=XH[:], in0=AGG[:], scalar=MU[:],
                in1=IV[:].to_broadcast([P, FE]), op0=AL.subtract,
                op1=AL.mult)
            nc.vector.tensor_tensor(out=XH[:], in0=XH[:], in1=GG_s[l][:],
                                    op=AL.mult)
            nc.vector.tensor_tensor(out=XH[:], in0=XH[:], in1=BE_s[l][:],
                                    op=AL.add)
            MN = npool.tile([P, FE], F32, tag='MN')
            nc.vector.tensor_scalar_min(MN[:], XH[:], 0.0)
            EX = npool.tile([P, FE], F32, tag='EX')
            nc.scalar.activation(out=EX[:], in_=MN[:], func=ACTF.Exp)
            RL = npool.tile([P, FE], F32, tag='RL')
            nc.vector.tensor_scalar_max(RL[:], XH[:], 0.0)
            hcol = h_res[l][:, b * FE:(b + 1) * FE]
            if l == 1:
                TMP = npool.tile([P, FE], F32, tag='TMP')
                nc.vector.scalar_tensor_tensor(
                    out=TMP[:], in0=EX[:], scalar=-1.0, in1=RL[:],
                    op0=AL.add, op1=AL.add)
                nc.vector.tensor_tensor(
                    out=hcol, in0=TMP[:],
                    in1=h_res[0][:, b * FE:(b + 1) * FE], op=AL.add)
            else:
                nc.vector.scalar_tensor_tensor(
                    out=hcol, in0=EX[:], scalar=-1.0, in1=RL[:],
                    op0=AL.add, op1=AL.add)

        def phase_b(l):
            # projections for layer l (1 or 2) from h_res[l-1]; xl rows to
            # ag_in[l], xr into xr_res[l]
            _, _, FE, _ = LCFG[l]
            C0 = 128 if l == 1 else 32
            hsrc = h_res[l - 1]
            for b in range(NBLK):
                psT = ps2pool.tile([P, P], F32, tag='psT')
                nc.tensor.transpose(out=psT[:],
                                    in_=hsrc[:, b * 128:(b + 1) * 128],
                                    identity=ident[:])
                hT = stpool.tile([P, P], F32, tag='hT')
                nc.scalar.copy(out=hT[:], in_=psT[:])
                ps = pspool.tile([P, 256], F32, tag='psA')
                wcols = 256 if l == 1 else 64
                nc.tensor.matmul(out=ps[:, 0:wcols], lhsT=hT[:],
                                 rhs=W01_s[l][:], start=True, stop=True)
                stg = stpool.tile([P, 128], BF16, tag='stgB')
                if l == 1:
                    nc.vector.tensor_tensor(out=stg[:], in0=ps[:, 0:128],
                                            in1=BL01_s[1][:, 0:128],
                                            op=AL.add)
                else:
                    nc.vector.memset(stg[:], 0)
                    nc.vector.tensor_tensor(out=stg[:, 0:32],
                                            in0=ps[:, 0:32],
                                            in1=BL01_s[2][:, 0:32],
                                            op=AL.add)
                if b == NBLK - 1:
                    nc.vector.tensor_scalar_mul(stg[:], stg[:], pmask_s[:])
                nc.sync.dma_start(out=ag_in[l][b * P:(b + 1) * P, :],
                                  in_=stg[:])
                nc.vector.tensor_tensor(
                    out=xr_res[l][:, b * FE:(b + 1) * FE],
                    in0=ps[:, C0:C0 + FE], in1=BL01_s[l][:, C0:C0 + FE],
                    op=AL.add)

        # ---------------- layer 0 ----------------
        for b in range(NBLK):
            node_tail(0, b, edge_block(0, b))
        # ---------------- layer 1 ----------------
        phase_b(1)
        nc.gpsimd.collective_compute(
            'AllGather', AL.bypass, replica_groups=[list(range(NC))],
            ins=[ag_in[1][:]], outs=[tabs[1][:]])
        for b in range(NBLK):
            node_tail(1, b, edge_block(1, b))
        # ---------------- layer 2 ----------------
        phase_b(2)
        nc.gpsimd.collective_compute(
            'AllGather', AL.bypass, replica_groups=[list(range(NC))],
            ins=[ag_in[2][:]], outs=[tabs[2][:]])
        for b in range(NBLK):
            node_tail(2, b, edge_block(2, b))
        # ---------------- MLP head ----------------
        for b in range(NBLK):
            psT = ps2pool.tile([P, P], F32, tag='psT')
            nc.tensor.transpose(out=psT[:HID, :],
                                in_=h_res[2][:, b * HID:(b + 1) * HID],
                                identity=ident[:])
            h2T = stpool.tile([HID, P], F32, tag='h2T')
            nc.scalar.copy(out=h2T[:], in_=psT[:HID, :])
            ps1 = pspool.tile([P, 16], F32, tag='psM')
            nc.tensor.matmul(out=ps1[:], lhsT=h2T[:], rhs=cW1_s[:],
                             start=True, stop=True)
            C1 = npool.tile([P, 16], F32, tag='C1')
            nc.vector.tensor_tensor(out=C1[:], in0=ps1[:], in1=cb1_s[:],
                                    op=AL.add)
            MN1 = npool.tile([P, 16], F32, tag='MN1')
            nc.vector.tensor_scalar_min(MN1[:], C1[:], 0.0)
            EX1 = npool.tile([P, 16], F32, tag='EX1')
            nc.scalar.activation(out=EX1[:], in_=MN1[:], func=ACTF.Exp)
            RL1 = npool.tile([P, 16], F32, tag='RL1')
            nc.vector.tensor_scalar_max(RL1[:], C1[:], 0.0)
            E1 = npool.tile([P, 16], F32, tag='E1')
            nc.vector.scalar_tensor_tensor(
                out=E1[:], in0=EX1[:], scalar=-1.0, in1=RL1[:],
                op0=AL.add, op1=AL.add)
            psT2 = ps2pool.tile([P, P], F32, tag='psT')
            nc.tensor.transpose(out=psT2[:16, :], in_=E1[:],
                                identity=ident[:])
            c1T = stpool.tile([16, P], F32, tag='c1T')
            nc.scalar.copy(out=c1T[:], in_=psT2[:16, :])
            ps2 = pspool.tile([P, 16], F32, tag='psM')
            nc.tensor.matmul(out=ps2[:, 0:1], lhsT=c1T[:], rhs=cW2_s[:],
                             start=True, stop=True)
            nc.vector.tensor_tensor(out=out_sb[:, b:b + 1],
                                    in0=ps2[:, 0:1], in1=cb2_s[:],
                                    op=AL.add)
        nc.sync.dma_start(out=out_d[:].rearrange('(b p) -> p b', p=P),
                          in_=out_sb[:])

    nc.compile()
    return nc


# ----------------------------------------------------------------------------
# entry point
# ----------------------------------------------------------------------------

def _make_in_maps(st, inputs, xT, xT_own, idx_all, mask_all, pm):
    Wl0 = np.asarray(inputs['Wl0'], np.float32)
    Wr0 = np.asarray(inputs['Wr0'], np.float32)
    Wl1 = np.asarray(inputs['Wl1'], np.float32)
    Wr1 = np.asarray(inputs['Wr1'], np.float32)
    Wl2 = np.asarray(inputs['Wl2'], np.float32)
    Wr2 = np.asarray(inputs['Wr2'], np.float32)
    shared = {
        'xT': xT,
        'W01_0': np.ascontiguousarray(np.concatenate([Wl0, Wr0], axis=1)),
        'W01_1': np.ascontiguousarray(np.concatenate([Wl1, Wr1], axis=1)),
        'W01_2': np.ascontiguousarray(np.concatenate([Wl2, Wr2], axis=1)),
        'bl01_0': _rep(np.concatenate([inputs['bl0'], inputs['br0']])),
        'bl01_1': _rep(np.concatenate([inputs['bl1'], inputs['br1']])),
        'bl01_2': _rep(np.concatenate([inputs['bl2'], inputs['br2']])),
        'att_0': _rep(np.asarray(inputs['att0']).reshape(-1)),
        'att_1': _rep(np.asarray(inputs['att1']).reshape(-1)),
        'att_2': _rep(np.asarray(inputs['att2']).reshape(-1)),
        'g_0': _rep(inputs['g0']), 'g_1': _rep(inputs['g1']),
        'g_2': _rep(inputs['g2']),
        'be_0': _rep(inputs['be0']), 'be_1': _rep(inputs['be1']),
        'be_2': _rep(inputs['be2']),
        'bo_0': _rep(inputs['bo0']), 'bo_1': _rep(inputs['bo1']),
        'bo_2': _rep(inputs['bo2']),
        'cW1': np.asarray(inputs['cW1'], np.float32),
        'cb1': _rep(inputs['cb1']),
        'cW2': np.asarray(inputs['cW2'], np.float32),
        'ident': np.eye(P, dtype=np.float32),
        'cb2': _rep(inputs['cb2']),
    }
    in_maps = []
    for c in range(NC):
        m = dict(shared)
        m['padmask'] = pm
        m['xT_own'] = xT_own[c]
        m['idx_all'] = idx_all[c]
        m['mask_all'] = mask_all[c]
        in_maps.append(m)
    return in_maps


_CACHE = {}


def _run_sim(nc, in_maps):
    from concourse.bass_interp import MultiCoreSim
    sim = MultiCoreSim(nc, num_cores=NC, trace=False,
                       require_finite=False, require_nnan=False)
    cores = list(sim.cores.values())
    for c in range(NC):
        for k, v in in_maps[c].items():
            cores[c].tensor(k)[:] = v
    sim.simulate(check_with_hw=False)
    return [{'out': np.array(cores[c].tensor('out'))} for c in range(NC)]


def kernel(trace=False, backend='hw', **inputs):
    x = np.asarray(inputs['x'], np.float32)
    (st, xT, xT_own, idx_all, mask_all, row,
     padmask) = _prep(x, inputs['edge_index'])
    key = (x.shape, np.asarray(inputs['edge_index']).shape)
    skey = str(sorted(st.items()))
    if skey not in _CACHE:
        _CACHE[skey] = _build(st)
    nc = _CACHE[skey]
    in_maps = _make_in_maps(st, inputs, xT, xT_own, idx_all, mask_all,
                            padmask)
    if backend == 'sim':
        results = _run_sim(nc, in_maps)
        res = None
    else:
        res = bass_utils.run_bass_kernel_spmd(
            nc, in_maps, core_ids=list(range(NC)), trace=trace)
        results = res.results
    cat = np.concatenate([results[c]['out'] for c in range(NC)])
    out = cat[row]
    if trace:
        kernel.last_results = res
    return out.astype(np.float32)



# revision 7
# speedup vs baseline: 1.0809x; 1.0809x over previous
"""Self-contained Trainium2 Bass kernel for nn_DualGATv2 (3-layer GATv2 + MLP).

Feature-major design: each destination block of 128 nodes keeps its edge
tensors as [feat(128 partitions), k-slot, dst(128)] so that
  - per-edge leaky_relu / adds / mults run contiguously on DVE in fp16 (2x/4x)
  - attention logits come from ONE tensor-engine matmul per psum chunk
    (contraction over features), replicated per-head across partitions
  - exp runs on the scalar engine straight out of PSUM
  - scatter-softmax denominator and weighted aggregation are in-place
    binary-tree adds over the k axis (contiguous halves, fp16 2x)
  - LayerNorm statistics are ones-matmuls; normalization is applied through
    rank-1 outer-product matmuls (g x rstd, be x 1 + g x (-mu rstd))
Nodes are degree-sorted and snake-dealt across 8 cores; small weights are
replicated; per-layer source features live in fp16 tables [TAB, 128]
(layer 0 computed locally from the replicated x; layers 1-2 via AllGather).
Per-edge rows are fetched with gpsimd dma_gather(transpose=True) so gathered
rows land as columns (int16 indices; lo/hi windows cover TAB > 32768 rows).
Pad slots gather a zeroed row; the softmax denominator is fixed up by
subtracting npad * exp(att . leaky(xr)) per node (no per-edge mask tensor).
"""
import sys
import numpy as np

sys.path.insert(0, '/opt/trn_rl_repo')

import concourse.bass as bass
import concourse.bacc as bacc
import concourse.tile as tile
from concourse import mybir, library_config
from concourse import bass_utils
from concourse._compat import cdiv

F32 = mybir.dt.float32
F16 = mybir.dt.float16
I16 = mybir.dt.int16
AL = mybir.AluOpType
ACTF = mybir.ActivationFunctionType
AX = mybir.AxisListType

NC = 8
P = 128
HID = 32
HEADS = 4
NEG_SLOPE = 0.2
LN_EPS = 1e-5
IDX_WIN = 32768

# tuning knobs
GMAX = 7          # max k-slots per dma_gather call
SINGLE_PACKET = True
CCH = 1024        # psum columns per logits/exp chunk (2 banks)
LNG = 7           # blocks per LayerNorm sqrt batch group


# ----------------------------------------------------------------------------
# host-side preprocessing (vectorized)
# ----------------------------------------------------------------------------

def _prep(x, edge_index):
    x = np.asarray(x, dtype=np.float32)
    N = x.shape[0]
    src = np.asarray(edge_index[0], dtype=np.int64)
    dst = np.asarray(edge_index[1], dtype=np.int64)
    loop = np.arange(N, dtype=np.int64)
    src = np.concatenate([src, loop])
    dst = np.concatenate([dst, loop])

    deg = np.bincount(dst, minlength=N)

    order = np.argsort(-deg, kind='stable')
    ranks = np.arange(N)
    g, j = ranks // NC, ranks % NC
    core_of_rank = np.where(g % 2 == 0, j, NC - 1 - j)
    core = np.zeros(N, dtype=np.int64)
    core[order] = core_of_rank
    pos = np.zeros(N, dtype=np.int64)
    pos[order] = ranks // NC

    NSH_REAL = cdiv(N, NC)
    NSH = cdiv(NSH_REAL + 1, P) * P
    NBLK = NSH // P
    TAB = NC * NSH
    row = core * NSH + pos
    W_LO = min(IDX_WIN, TAB)
    HI_BASE = max(0, TAB - IDX_WIN)
    ZR_LO = NSH - 1
    ZR_HI = TAB - 1 - HI_BASE

    e_order = np.argsort(dst, kind='stable')
    src_s = src[e_order]
    dst_s = dst[e_order]
    rs_all = row[src_s]
    E2 = len(dst_s)
    starts = np.searchsorted(dst_s, np.arange(N))

    forcedA = rs_all < HI_BASE
    forcedB = rs_all >= W_LO
    flex = ~forcedA & ~forcedB
    nAf = np.bincount(dst_s, weights=forcedA, minlength=N).astype(np.int64)
    nBf = np.bincount(dst_s, weights=forcedB, minlength=N).astype(np.int64)
    d = deg
    ca = np.minimum(np.maximum((d + 1) // 2, nAf), d - nBf)

    start_of_e = starts[dst_s]
    cflex = np.cumsum(flex)
    c0 = np.where(start_of_e > 0, cflex[np.maximum(start_of_e - 1, 0)], 0)
    flexrank = cflex - flex - c0
    selA = forcedA | (flex & (flexrank < (ca - nAf)[dst_s]))

    cselA = np.cumsum(selA)
    c0A = np.where(start_of_e > 0, cselA[np.maximum(start_of_e - 1, 0)], 0)
    kA = cselA - selA - c0A
    selB = ~selA
    cselB = np.cumsum(selB)
    c0B = np.where(start_of_e > 0, cselB[np.maximum(start_of_e - 1, 0)], 0)
    kB = cselB - selB - c0B

    cA = np.bincount(dst_s, weights=selA, minlength=N).astype(np.int64)
    cB = d - cA

    b_of = pos // P          # block of node
    p_of = pos % P           # lane of node
    K_A = np.zeros(NBLK, dtype=np.int64)
    K_B = np.zeros(NBLK, dtype=np.int64)
    np.maximum.at(K_A, b_of, cA)
    np.maximum.at(K_B, b_of, cB)
    K_A = np.maximum(K_A, 1)
    K_B = np.maximum(K_B, 1)
    SUMKT = int((K_A + K_B).sum())

    off_b = np.zeros(NBLK + 1, dtype=np.int64)
    off_b[1:] = np.cumsum(K_A + K_B)

    # flat column index inside the per-core idx stream (in slots of 128):
    # window A of block b occupies slots [off_b[b], off_b[b]+K_A[b])
    # window B occupies [off_b[b]+K_A[b], off_b[b+1])
    c_e = core[dst_s]
    b_e = b_of[dst_s]
    p_e = p_of[dst_s]
    colA = (off_b[b_e] + kA) * P + p_e
    colB = (off_b[b_e] + K_A[b_e] + kB) * P + p_e

    TOT = SUMKT * P
    flat_idx = np.empty((NC, TOT), dtype=np.int64)
    # defaults: pad slots point at the zero rows of each window
    defA = np.zeros(TOT, dtype=bool)
    for b in range(NBLK):
        defA[off_b[b] * P:(off_b[b] + K_A[b]) * P] = True
    flat_idx[:, :] = np.where(defA, ZR_LO, ZR_HI)[None, :]
    eA = selA.nonzero()[0]
    flat_idx[c_e[eA], colA[eA]] = rs_all[eA]
    eB = selB.nonzero()[0]
    flat_idx[c_e[eB], colB[eB]] = rs_all[eB] - HI_BASE

    # wrap into the 16-channel idx layout replicated across 8 q7 cores
    S = TOT // 16
    idx_all = np.empty((NC, 128, S), dtype=np.int16)
    for c in range(NC):
        w = flat_idx[c].astype(np.int16).reshape(S, 16).T
        idx_all[c] = np.tile(w, (8, 1))

    # npad per (core, block, lane), replicated across feature partitions
    kt_of = (K_A + K_B)
    npad = np.empty((NC, NBLK * P), dtype=np.float32)
    npad[:, :] = np.repeat(kt_of, P)[None, :]
    npad[core, pos + 0] = (kt_of[b_of] - d)
    npad_rep = np.ascontiguousarray(
        np.broadcast_to(npad[:, None, :], (NC, 128, NBLK * P))).astype(np.float16)

    IND = x.shape[1]
    xT = np.zeros((IND, TAB), dtype=np.float16)
    xT[:, row] = x.T.astype(np.float16)
    xT_own = np.ascontiguousarray(
        xT.reshape(IND, NC, NSH).transpose(1, 0, 2))

    nlast = NSH_REAL - (NBLK - 1) * P
    pmask_col = (np.arange(P) < nlast).astype(np.float32).reshape(P, 1)
    pmask_rep = np.ascontiguousarray(np.broadcast_to(
        (np.arange(P) < nlast).astype(np.float16).reshape(1, P), (P, P)))

    st = dict(N=N, NSH=NSH, NSH_REAL=NSH_REAL, NBLK=NBLK, TAB=TAB,
              W_LO=W_LO, HI_BASE=HI_BASE, K_A=K_A.tolist(),
              K_B=K_B.tolist(), SUMKT=SUMKT, IN_DIM=IND)
    return st, xT, xT_own, idx_all, npad_rep, row, pmask_col, pmask_rep


def _rep(v, parts=P):
    v = np.asarray(v, dtype=np.float32).reshape(1, -1)
    return np.ascontiguousarray(np.tile(v, (parts, 1)))


# ----------------------------------------------------------------------------
# kernel builder
# ----------------------------------------------------------------------------

def _build(st):
    NSH, NBLK, TAB = st['NSH'], st['NBLK'], st['TAB']
    W_LO, HI_BASE = st['W_LO'], st['HI_BASE']
    K_A, K_B = st['K_A'], st['K_B']
    SUMKT = st['SUMKT']
    IND = st['IN_DIM']

    # per-layer: (KP feature partitions of this layer's edge tensors,
    #             FE output width, input width)
    LCFG = [(128, 128), (128, 128), (32, 32)]

    off_b = [0]
    for b in range(NBLK):
        off_b.append(off_b[-1] + K_A[b] + K_B[b])

    nc = bacc.Bacc('TRN2', target_bir_lowering=False, debug=False,
                   enable_asserts=True, num_devices=NC,
                   num_swdge_queues=4)

    def ein(name, shape, dt=F32):
        return nc.dram_tensor(name, shape, dt, kind='ExternalInput')

    xT_d = ein('xT', [IND, TAB], F16)
    xTo_d = ein('xT_own', [IND, NSH], F16)
    idx_d = ein('idx_all', [P, 8 * SUMKT], I16)
    npad_d = ein('npad_rep', [P, NBLK * P], F16)
    pmc_d = ein('pmask_col', [P, 1])
    pmr_d = ein('pmask_rep', [P, P], F16)
    Wl_d = [ein('Wl0', [IND, 128], F16), ein('Wl1', [128, 128], F16),
            ein('Wl2', [128, 32], F16)]
    Wr_d = [ein('Wr0', [IND, 128], F16), ein('Wr1', [128, 128], F16),
            ein('Wr2', [128, 32], F16)]
    blr_d = [ein('blr0', [P, 128], F16), ein('blr1', [P, 128], F16),
             ein('blr2', [P, 32], F16)]
    br_d = [ein('br0', [128, 1]), ein('br1', [128, 1]), ein('br2', [32, 1])]
    bo_d = [ein('bo0', [128, 1]), ein('bo1', [128, 1]), ein('bo2', [32, 1])]
    att_d = [ein('attr0', [128, 128], F16), ein('attr1', [128, 128], F16),
             ein('attr2', [32, 32], F16)]
    grow_d = [ein('grow0', [1, 128]), ein('grow1', [1, 128]),
              ein('grow2', [1, 32])]
    berow_d = [ein('berow0', [1, 128]), ein('berow1', [1, 128]),
               ein('berow2', [1, 32])]
    onesr_d = ein('ones_row', [1, 128])
    ones_d = ein('ones_col', [P, 1], F16)
    cW1_d = ein('cW1', [32, 16], F16)
    cb1_d = ein('cb1', [16, 1])
    cW2_d = ein('cW2', [16, 1], F16)
    cb2_d = ein('cb2', [1, 1])
    out_d = nc.dram_tensor('out', [NSH], F32, kind='ExternalOutput')

    tabs = [nc.dram_tensor('table0', [TAB, 128], F16, kind='Internal'),
            nc.dram_tensor('table1', [TAB, 128], F16, kind='Internal',
                           addr_space='Shared'),
            nc.dram_tensor('table2', [TAB, 128], F16, kind='Internal',
                           addr_space='Shared')]
    ag_in = [None,
             nc.dram_tensor('ag_in1', [NSH, 128], F16, kind='Internal'),
             nc.dram_tensor('ag_in2', [NSH, 128], F16, kind='Internal')]

    import contextlib
    with tile.TileContext(nc) as tc, contextlib.ExitStack() as ctx:
        cpool = ctx.enter_context(tc.tile_pool(name='consts', bufs=1))
        epool = ctx.enter_context(tc.tile_pool(name='edge', bufs=2))
        xpool = ctx.enter_context(tc.tile_pool(name='xchunk', bufs=2))
        spool = ctx.enter_context(tc.tile_pool(name='small', bufs=2))
        gpool = ctx.enter_context(tc.tile_pool(name='grp', bufs=2))
        stpool = ctx.enter_context(tc.tile_pool(name='stage', bufs=2))
        hpool = ctx.enter_context(tc.tile_pool(name='resid', bufs=1))
        pse_pool = ctx.enter_context(tc.tile_pool(name='pse', bufs=2,
                                                  space='PSUM'))
        psm_pool = ctx.enter_context(tc.tile_pool(name='psm', bufs=4,
                                                  space='PSUM'))

        def load_const(dram, dt=None):
            shape = list(dram.shape)
            t = cpool.tile(shape, dt or dram.dtype, tag='c_' + dram.name,
                           name='c_' + dram.name)
            nc.sync.dma_start(out=t[:], in_=dram[:])
            return t

        Wl_s = [load_const(w) for w in Wl_d]
        Wr_s = [load_const(w) for w in Wr_d]
        blr_s = [load_const(w) for w in blr_d]
        br_s = [load_const(w) for w in br_d]
        bo_s = [load_const(w) for w in bo_d]
        att_s = [load_const(w) for w in att_d]
        grow_s = [load_const(w) for w in grow_d]
        berow_s = [load_const(w) for w in berow_d]
        onesr_s = load_const(onesr_d)
        ones_s = load_const(ones_d)
        cW1_s = load_const(cW1_d)
        cb1_s = load_const(cb1_d)
        cW2_s = load_const(cW2_d)
        cb2_s = load_const(cb2_d)
        pmc_s = load_const(pmc_d)
        pmr_s = load_const(pmr_d)
        idx_s = cpool.tile([P, 8 * SUMKT], I16, tag='idx')
        nc.sync.dma_start(out=idx_s[:], in_=idx_d[:])
        eps_t = cpool.tile([1, 1], F32, tag='eps', name='eps')
        nc.vector.memset(eps_t[:], float(LN_EPS))
        npad_s = cpool.tile([P, NBLK * P], F16, tag='npad')
        nc.sync.dma_start(out=npad_s[:], in_=npad_d[:])

        xr_ping = hpool.tile([128, NBLK * P], F16, tag='xrA', name='xrA')
        xr_pong = hpool.tile([128, NBLK * P], F16, tag='xrB', name='xrB')
        xr_t = [xr_ping, xr_pong, xr_ping]
        h0_t = hpool.tile([128, NBLK * P], F16, tag='h0', name='h0')

        qc = [0]

        # ------------- layer-0 projections (x replicated => local) -------
        # xr0 (own shard, feature-major)
        for g0 in range(0, NBLK, LNG):
            nb = min(LNG, NBLK - g0)
            xoch = xpool.tile([IND, LNG * P], F16, tag='xchunk',
                              name='xoch')
            nc.sync.dma_start(out=xoch[:, 0:nb * P],
                              in_=xTo_d[:, g0 * P:(g0 + nb) * P])
            for jj in range(nb):
                b = g0 + jj
                psq = psm_pool.tile([128, P], F32, tag='psm')
                nc.tensor.matmul(out=psq[:], lhsT=Wr_s[0][:],
                                 rhs=xoch[:, jj * P:(jj + 1) * P],
                                 start=True, stop=True)
                nc.scalar.activation(out=xr_t[0][:, b * P:(b + 1) * P],
                                     in_=psq[:], func=ACTF.Identity,
                                     bias=br_s[0][:])
        # table0 rows (all cores compute full table locally)
        for c in range(NC):
            for gblk in range(0, NBLK, LNG):
                nb = min(LNG, NBLK - gblk)
                xch = xpool.tile([IND, LNG * P], F16, tag='xchunk',
                                 name='xch')
                nc.sync.dma_start(
                    out=xch[:, 0:nb * P],
                    in_=xT_d[:, c * NSH + gblk * P:
                             c * NSH + (gblk + nb) * P])
                stg = stpool.tile([P, LNG * 128], F16, tag='stg')
                for t in range(nb):
                    b = gblk + t
                    psp = psm_pool.tile([128, 128], F32, tag='psm')
                    nc.tensor.matmul(out=psp[:],
                                     lhsT=xch[:, t * P:(t + 1) * P],
                                     rhs=Wl_s[0][:], start=True, stop=True)
                    sl = stg[:, t * 128:(t + 1) * 128]
                    nc.vector.tensor_tensor(out=sl, in0=psp[:],
                                            in1=blr_s[0][:], op=AL.add)
                    if b == NBLK - 1:
                        nc.vector.tensor_scalar_mul(sl, sl, pmc_s[:])
                base = (c * NBLK + gblk) * P
                dst_ap = tabs[0][base:base + nb * P, :] \
                    .rearrange('(t p) f -> p t f', p=P)
                nc.sync.dma_start(
                    out=dst_ap,
                    in_=stg[:, 0:nb * 128].rearrange('p (t f) -> p t f',
                                                     f=128))

        def tree_add(tile_ap, kt, KP):
            v = tile_ap.rearrange('p (k n) -> p k n', k=kt)
            n = kt
            while n > 1:
                h = n // 2
                nc.vector.tensor_tensor(out=v[0:KP, 0:h, :],
                                        in0=v[0:KP, 0:h, :],
                                        in1=v[0:KP, n - h:n, :], op=AL.add)
                n = n - h

        def edge_phase(l, b, XXg, mu_g, vpe_g, jj):
            KP, FE = LCFG[l]
            ka, kb = K_A[b], K_B[b]
            kt = ka + kb
            ic = off_b[b]
            ga = epool.tile([128, kt * 128], F16, tag='GA')
            for off in range(0, ka, GMAX):
                kk = min(GMAX, ka - off)
                nc.gpsimd.dma_gather(
                    ga[:, (off) * 128:(off + kk) * 128]
                    .rearrange('p (o n) -> p o n', o=1),
                    tabs[l][0:W_LO, :],
                    idx_s[:, 8 * (ic + off):8 * (ic + off + kk)],
                    kk * P, kk * P, 128, transpose=True,
                    single_packet=SINGLE_PACKET, queue_num=qc[0] % 4)
                qc[0] += 1
            for off in range(0, kb, GMAX):
                kk = min(GMAX, kb - off)
                nc.gpsimd.dma_gather(
                    ga[:, (ka + off) * 128:(ka + off + kk) * 128]
                    .rearrange('p (o n) -> p o n', o=1),
                    tabs[l][HI_BASE:TAB, :],
                    idx_s[:, 8 * (ic + ka + off):8 * (ic + ka + off + kk)],
                    kk * P, kk * P, 128, transpose=True,
                    single_packet=SINGLE_PACKET, queue_num=qc[0] % 4)
                qc[0] += 1
            xrb = xr_t[l][0:KP, b * P:(b + 1) * P]
            T = epool.tile([KP, kt * 128], F16, tag='T')
            T3 = T[:].rearrange('p (k n) -> p k n', k=kt)
            nc.vector.tensor_tensor(
                out=T3, in0=ga[0:KP, :].rearrange('p (k n) -> p k n', k=kt),
                in1=xrb.unsqueeze(1).to_broadcast([KP, kt, 128]), op=AL.add)
            nc.vector.scalar_tensor_tensor(
                out=T[:], in0=T[:], scalar=float(NEG_SLOPE), in1=T[:],
                op0=AL.mult, op1=AL.max)
            # logits chunks; exp overwrites T (leaky values die per chunk)
            for c0 in range(0, kt * 128, CCH):
                cw = min(CCH, kt * 128 - c0)
                pse = pse_pool.tile([KP, CCH], F32, tag='pse')
                for m0 in range(0, cw, 512):
                    mw = min(512, cw - m0)
                    nc.tensor.matmul(out=pse[:, m0:m0 + mw],
                                     lhsT=att_s[l][:],
                                     rhs=T[:, c0 + m0:c0 + m0 + mw],
                                     start=True, stop=True)
                nc.scalar.activation(out=T[:, c0:c0 + cw], in_=pse[:, 0:cw],
                                     func=ACTF.Exp)
            # W = GA * E (E lives in T now); W overwrites GA
            nc.vector.tensor_tensor(out=ga[0:KP, :], in0=ga[0:KP, :],
                                    in1=T[:], op=AL.mult)
            # denominator fixup ingredients
            pxr = spool.tile([KP, 128], F16, tag='pxr')
            nc.vector.scalar_tensor_tensor(
                out=pxr[:], in0=xrb, scalar=float(NEG_SLOPE), in1=xrb,
                op0=AL.mult, op1=AL.max)
            ps0 = psm_pool.tile([KP, 128], F32, tag='psm')
            nc.tensor.matmul(out=ps0[:], lhsT=att_s[l][:], rhs=pxr[:],
                             start=True, stop=True)
            e0 = spool.tile([KP, 128], F16, tag='e0')
            nc.scalar.activation(out=e0[:], in_=ps0[:], func=ACTF.Exp)
            # trees: D over E(=T), U over W(=GA)
            tree_add(T[:], kt, KP)
            tree_add(ga[0:KP, :], kt, KP)
            tmp = spool.tile([KP, 128], F16, tag='dtmp')
            nc.vector.tensor_tensor(out=tmp[:], in0=e0[:],
                                    in1=npad_s[0:KP, b * P:(b + 1) * P],
                                    op=AL.mult)
            dd = spool.tile([KP, 128], F32, tag='dd')
            nc.vector.tensor_tensor(out=dd[:], in0=T[:, 0:128], in1=tmp[:],
                                    op=AL.subtract)
            nc.vector.tensor_scalar_max(dd[:], dd[:], 1e-12)
            rr = spool.tile([KP, 128], F32, tag='rr')
            nc.vector.reciprocal_approx_fast(out=rr[:], in_=dd[:])
            ur = spool.tile([KP, 128], F32, tag='ur')
            nc.vector.tensor_tensor(out=ur[:], in0=ga[0:KP, 0:128],
                                    in1=rr[:], op=AL.mult)
            XXb = XXg[0:KP, jj * 128:(jj + 1) * 128]
            nc.scalar.activation(out=XXb, in_=ur[:], func=ACTF.Identity,
                                 bias=bo_s[l][:])
            x2 = spool.tile([KP, 128], F16, tag='x2')
            nc.scalar.activation(out=x2[:], in_=XXb, func=ACTF.Square)
            pss = psm_pool.tile([1, 256], F32, tag='psm')
            nc.tensor.matmul(out=pss[:, 0:128], lhsT=ones_s[0:KP, :],
                             rhs=XXb, start=True, stop=True)
            nc.tensor.matmul(out=pss[:, 128:256], lhsT=ones_s[0:KP, :],
                             rhs=x2[:], start=True, stop=True)
            nc.scalar.activation(out=mu_g[:, jj * 128:(jj + 1) * 128],
                                 in_=pss[:, 0:128], func=ACTF.Copy,
                                 scale=1.0 / FE)
            m2 = spool.tile([1, 128], F32, tag='m2')
            nc.scalar.activation(out=m2[:],
                                 in_=mu_g[:, jj * 128:(jj + 1) * 128],
                                 func=ACTF.Square)
            nc.vector.scalar_tensor_tensor(
                out=vpe_g[:, jj * 128:(jj + 1) * 128], in0=pss[:, 128:256],
                scalar=1.0 / FE, in1=m2[:], op0=AL.mult, op1=AL.subtract)

        def norm_proj(l, b, XXg, mu_g, rstd_g, jj, stg):
            KP, FE = LCFG[l]
            XXb = XXg[0:KP, jj * 128:(jj + 1) * 128]
            mub = mu_g[:, jj * 128:(jj + 1) * 128]
            rsb = rstd_g[:, jj * 128:(jj + 1) * 128]
            negmur = spool.tile([1, 128], F32, tag='negmur')
            nc.vector.scalar_tensor_tensor(
                out=negmur[:], in0=mub, scalar=-1.0, in1=rsb,
                op0=AL.mult, op1=AL.mult)
            psn1 = psm_pool.tile([KP, 128], F32, tag='psm')
            nc.tensor.matmul(out=psn1[:], lhsT=grow_s[l][:], rhs=rsb,
                             start=True, stop=True)
            psn2 = psm_pool.tile([KP, 128], F32, tag='psm')
            nc.tensor.matmul(out=psn2[:], lhsT=berow_s[l][:],
                             rhs=onesr_s[:], start=True, stop=False)
            nc.tensor.matmul(out=psn2[:], lhsT=grow_s[l][:],
                             rhs=negmur[:], start=False, stop=True)
            xn = spool.tile([KP, 128], F32, tag='xn')
            nc.vector.tensor_tensor(out=xn[:], in0=XXb, in1=psn1[:],
                                    op=AL.mult)
            xn2 = spool.tile([KP, 128], F16, tag='xn2')
            nc.vector.tensor_tensor(out=xn2[:], in0=xn[:], in1=psn2[:],
                                    op=AL.add)
            mn = spool.tile([KP, 128], F16, tag='mn')
            nc.vector.tensor_scalar_min(mn[:], xn2[:], 0.0)
            ex = spool.tile([KP, 128], F16, tag='ex')
            nc.scalar.activation(out=ex[:], in_=mn[:], func=ACTF.Exp)
            rl = spool.tile([KP, 128], F16, tag='rl')
            nc.vector.tensor_scalar_max(rl[:], xn2[:], 0.0)
            if l == 0:
                hb = h0_t[:, b * P:(b + 1) * P]
                nc.vector.scalar_tensor_tensor(
                    out=hb, in0=ex[:], scalar=-1.0, in1=rl[:],
                    op0=AL.add, op1=AL.add)
            else:
                hbt = spool.tile([KP, 128], F16, tag='hb')
                hb = hbt[:]
                if l == 1:
                    he = spool.tile([KP, 128], F16, tag='he')
                    nc.vector.scalar_tensor_tensor(
                        out=he[:], in0=ex[:], scalar=-1.0, in1=rl[:],
                        op0=AL.add, op1=AL.add)
                    nc.vector.tensor_tensor(out=hb, in0=he[:],
                                            in1=h0_t[:, b * P:(b + 1) * P],
                                            op=AL.add)
                else:
                    nc.vector.scalar_tensor_tensor(
                        out=hb, in0=ex[:], scalar=-1.0, in1=rl[:],
                        op0=AL.add, op1=AL.add)
            if b == NBLK - 1:
                nc.vector.tensor_tensor(out=hb, in0=hb, in1=pmr_s[0:KP, :],
                                        op=AL.mult)
            if l < 2:
                KP2, FE2 = LCFG[l + 1]
                psp = psm_pool.tile([128, FE2], F32, tag='psm')
                nc.tensor.matmul(out=psp[:], lhsT=hb, rhs=Wl_s[l + 1][:],
                                 start=True, stop=True)
                sl = stg[:, jj * FE2:(jj + 1) * FE2]
                nc.vector.tensor_tensor(out=sl, in0=psp[:],
                                        in1=blr_s[l + 1][:], op=AL.add)
                if b == NBLK - 1:
                    nc.vector.tensor_scalar_mul(sl, sl, pmc_s[:])
                psq = psm_pool.tile([KP2, 128], F32, tag='psm')
                nc.tensor.matmul(out=psq[:], lhsT=Wr_s[l + 1][:], rhs=hb,
                                 start=True, stop=True)
                nc.scalar.activation(
                    out=xr_t[l + 1][0:KP2, b * P:(b + 1) * P],
                    in_=psq[:], func=ACTF.Identity, bias=br_s[l + 1][:])
            else:
                psm1 = psm_pool.tile([16, 128], F32, tag='psm')
                nc.tensor.matmul(out=psm1[:], lhsT=cW1_s[:], rhs=hb,
                                 start=True, stop=True)
                c1 = spool.tile([16, 128], F32, tag='c1')
                nc.scalar.activation(out=c1[:], in_=psm1[:],
                                     func=ACTF.Identity, bias=cb1_s[:])
                mn1 = spool.tile([16, 128], F16, tag='mn1')
                nc.vector.tensor_scalar_min(mn1[:], c1[:], 0.0)
                ex1 = spool.tile([16, 128], F16, tag='ex1')
                nc.scalar.activation(out=ex1[:], in_=mn1[:], func=ACTF.Exp)
                rl1 = spool.tile([16, 128], F16, tag='rl1')
                nc.vector.tensor_scalar_max(rl1[:], c1[:], 0.0)
                e1 = spool.tile([16, 128], F16, tag='e1')
                nc.vector.scalar_tensor_tensor(
                    out=e1[:], in0=ex1[:], scalar=-1.0, in1=rl1[:],
                    op0=AL.add, op1=AL.add)
                psm2 = psm_pool.tile([1, 128], F32, tag='psm')
                nc.tensor.matmul(out=psm2[:], lhsT=cW2_s[:], rhs=e1[:],
                                 start=True, stop=True)
                ob = spool.tile([1, 128], F32, tag='ob')
                nc.scalar.activation(out=ob[:], in_=psm2[:],
                                     func=ACTF.Identity, bias=cb2_s[:])
                nc.sync.dma_start(
                    out=out_d[b * P:(b + 1) * P]
                    .rearrange('(a n) -> a n', a=1), in_=ob[:])

        for l in range(3):
            KP, FE = LCFG[l]
            FE2 = LCFG[l + 1][1] if l < 2 else 0
            with nc.allow_low_precision(reason='fp16 edge softmax trees'):
                for g0 in range(0, NBLK, LNG):
                    nb = min(LNG, NBLK - g0)
                    XXg = gpool.tile([KP, LNG * 128], F16, tag='XXg')
                    mu_g = gpool.tile([1, LNG * 128], F32, tag='mug')
                    vpe_g = gpool.tile([1, LNG * 128], F32, tag='vpeg')
                    for jj in range(nb):
                        edge_phase(l, g0 + jj, XXg, mu_g, vpe_g, jj)
                    sd_g = gpool.tile([1, LNG * 128], F32, tag='sdg')
                    nc.scalar.activation(out=sd_g[:, 0:nb * 128],
                                         in_=vpe_g[:, 0:nb * 128],
                                         func=ACTF.Sqrt, bias=eps_t[:])
                    rstd32 = gpool.tile([1, LNG * 128], F32, tag='rstd32')
                    nc.vector.reciprocal_approx_fast(
                        out=rstd32[:, 0:nb * 128], in_=sd_g[:, 0:nb * 128])
                    if l < 2:
                        stg = stpool.tile([P, LNG * FE2], F16, tag='stg',
                                          name='stg')
                    else:
                        stg = None
                    for jj in range(nb):
                        norm_proj(l, g0 + jj, XXg, mu_g, rstd32, jj, stg)
                    if l < 2:
                        base = g0 * P
                        dst_ap = ag_in[l + 1][base:base + nb * P, 0:FE2] \
                            .rearrange('(t p) f -> p t f', p=P)
                        nc.sync.dma_start(
                            out=dst_ap,
                            in_=stg[:, 0:nb * FE2]
                            .rearrange('p (t f) -> p t f', f=FE2))
            if l < 2:
                nc.gpsimd.collective_compute(
                    'AllGather', AL.bypass,
                    replica_groups=[list(range(NC))],
                    ins=[ag_in[l + 1][:]], outs=[tabs[l + 1][:]])

    nc.compile()
    return nc


# ----------------------------------------------------------------------------
# entry point
# ----------------------------------------------------------------------------

def _make_in_maps(st, inputs, xT, xT_own, idx_all, npad_rep, pmask_col,
                  pmask_rep):
    def f16(a):
        return np.ascontiguousarray(np.asarray(a, np.float32)
                                    .astype(np.float16))

    def col(v, parts):
        return np.ascontiguousarray(
            np.asarray(v, np.float32).reshape(-1, 1)[:parts])

    att_rep = []
    for l, parts in ((0, 128), (1, 128), (2, 32)):
        att = np.asarray(inputs[f'att{l}'], np.float32)  # [h, dout]
        h, dout = att.shape
        A = np.zeros((parts, h * dout), dtype=np.float32)
        for hh in range(h):
            A[hh * dout:(hh + 1) * dout, hh * dout:(hh + 1) * dout] = \
                np.tile(att[hh].reshape(dout, 1), (1, dout))
        att_rep.append(f16(A))

    shared = {
        'xT': xT,
        'Wl0': f16(inputs['Wl0']), 'Wl1': f16(inputs['Wl1']),
        'Wl2': f16(inputs['Wl2']),
        'Wr0': f16(inputs['Wr0']), 'Wr1': f16(inputs['Wr1']),
        'Wr2': f16(inputs['Wr2']),
        'blr0': f16(_rep(inputs['bl0'])), 'blr1': f16(_rep(inputs['bl1'])),
        'blr2': f16(_rep(inputs['bl2'])),
        'br0': col(inputs['br0'], 128), 'br1': col(inputs['br1'], 128),
        'br2': col(inputs['br2'], 32),
        'bo0': col(inputs['bo0'], 128), 'bo1': col(inputs['bo1'], 128),
        'bo2': col(inputs['bo2'], 32),
        'attr0': att_rep[0], 'attr1': att_rep[1], 'attr2': att_rep[2],
        'grow0': np.asarray(inputs['g0'], np.float32).reshape(1, -1),
        'grow1': np.asarray(inputs['g1'], np.float32).reshape(1, -1),
        'grow2': np.asarray(inputs['g2'], np.float32).reshape(1, -1),
        'berow0': np.asarray(inputs['be0'], np.float32).reshape(1, -1),
        'berow1': np.asarray(inputs['be1'], np.float32).reshape(1, -1),
        'berow2': np.asarray(inputs['be2'], np.float32).reshape(1, -1),
        'ones_row': np.ones((1, 128), np.float32),
        'ones_col': f16(np.ones((P, 1), np.float32)),
        'cW1': f16(inputs['cW1']), 'cb1': col(inputs['cb1'], 16),
        'cW2': f16(inputs['cW2']), 'cb2': col(inputs['cb2'], 1),
        'pmask_col': pmask_col,
        'pmask_rep': pmask_rep,
    }
    in_maps = []
    for c in range(NC):
        m = dict(shared)
        m['xT_own'] = xT_own[c]
        m['idx_all'] = idx_all[c]
        m['npad_rep'] = npad_rep[c]
        in_maps.append(m)
    return in_maps


_CACHE = {}


def _run_sim(nc, in_maps):
    from concourse.bass_interp import MultiCoreSim
    sim = MultiCoreSim(nc, num_cores=NC, trace=False,
                       require_finite=False, require_nnan=False)
    cores = list(sim.cores.values())
    for c in range(NC):
        for k, v in in_maps[c].items():
            cores[c].tensor(k)[:] = v
    sim.simulate(check_with_hw=False)
    return [{'out': np.array(cores[c].tensor('out'))} for c in range(NC)]


def kernel(trace=False, backend='hw', **inputs):
    x = np.asarray(inputs['x'], np.float32)
    (st, xT, xT_own, idx_all, npad_rep, row, pmask_col,
     pmask_rep) = _prep(x, inputs['edge_index'])
    skey = str(sorted((k, v) for k, v in st.items()
               if not isinstance(v, list))) + str(st['K_A']) + str(st['K_B'])
    if skey not in _CACHE:
        _CACHE[skey] = _build(st)
    nc = _CACHE[skey]
    in_maps = _make_in_maps(st, inputs, xT, xT_own, idx_all, npad_rep,
                            pmask_col, pmask_rep)
    if backend == 'sim':
        results = _run_sim(nc, in_maps)
        res = None
    else:
        res = bass_utils.run_bass_kernel_spmd(
            nc, in_maps, core_ids=list(range(NC)), trace=trace)
        results = res.results
    cat = np.concatenate([results[c]['out'] for c in range(NC)])
    out = cat[row]
    if trace:
        kernel.last_results = res
    return out.astype(np.float32)
